# revision 1
# baseline (speedup 1.0000x reference)
"""CIN (Compressed Interaction Network) forward kernel for Trainium2.

Computation (per reference):
  z0 = relu(einsum('bid,bjd,ijm->bmd', x, x,  W0))   W0: (39,39,128)
  h1 = z0[:, :64];  fin0 = z0[:, 64:]
  z1 = relu(einsum('bid,bjd,ijm->bmd', x, h1, W1))   W1: (39,64,128)
  out = concat([fin0, z1], 1).sum(-1) @ dense_w + dense_b

Strategy: pure data-parallel over batch (4096 -> 8 cores x 512); per core
columns n = (b, d) = 8192, software-pipelined over column tiles so layer 0
of tile t overlaps layer 1 of tile t-1.
- Layer 0 exploits einsum symmetry: W0 folded to 20 offset-diagonals
  (K 1521 -> 780); outer-product operands are contiguous slices of [x; x],
  so no partition replication is needed (7 chunks of <=117 rows).
- Layer 1 blocks (i,j) as 8x16; the X side is DMA-replicated via repeat
  access patterns, the h1 side round-trips through a DRAM scratch so each
  replicated tile is ONE dma_start (HWDGE fixed cost ~600ns/start).
- Outer products on DVE (3/4) + GpSimd (1/4); contraction on the
  TensorEngine with PSUM accumulation; relu on ScalarE; dense layer folded
  into a PE matvec + DVE grouped d-reduce. All bf16, fp32 PSUM.
- Fill-phase x-operands for tile 0 are host-prepacked (xpk/xrpk) and
  loaded in 3 contiguous wide-descriptor DMAs instead of 13 replicate-AP
  starts: the fill is bound by HWDGE start serialization (~650ns each).
- Tile widths [704, 1280, 1408, 1536, 1472, 1280, 512] tuned against the
  cost-model timeline sim: graduated fill (DMA-latency-bound), wide steady
  state (engines ~saturated and balanced: PE 101us, DVE ~99, Pool ~85,
  DMA ~99 of 132.6us total), 512 drain tile.  The engine layout is a tight
  local optimum: coarser product ops, batched DMA starts, SBUF-sourced h
  replication, and deeper pipelining all measurably regress (PE p-state
  ramp resets + serialized DMA-queue FIFO effects punish any added
  latency in the z0 -> relu -> h-replicate -> products chain).
"""
import numpy as np
import ml_dtypes

import concourse.bass as bass
import concourse.bacc as bacc
import concourse.mybir as mybir
from concourse.alu_op_type import AluOpType
from concourse.tile import TileContext
from concourse.bass_utils import run_bass_kernel_spmd

BF16 = mybir.dt.bfloat16
F32 = mybir.dt.float32
B, F0, D = 4096, 39, 16
NCORES = 8
BC = B // NCORES            # batch per core
N = BC * D                  # columns per core
NT = 2048                   # column tile width
FK1 = 64                    # layer-1 hidden field count
A, G = 8, 16                # i-block, j-block sizes
NBI = 5                     # i-blocks over 39 (pad to 40)
NBJ1 = 4                    # layer-1 j-blocks over 64
NCH0 = 7                    # layer-0 folded diag chunks (20 offsets x 39 rows)
XROWS = 48                  # padded x rows
FILLW0 = 704                # tile-0 width (packed x-operand block)
FILLW1 = 1280               # tile-1 width (packed x-operand block)


def _build(n=N, widths=None, tt_engines=("vector", "vector", "vector", "gpsimd", "vector", "vector", "vector", "vector", "gpsimd"), bufs=None):
    """Software-pipelined builder: L0(t) overlaps L1(t-1)."""
    if widths is None:
        widths = [704, 1280, 1408, 1536, 1472, 1280, 512] if n == 8192 else [min(n, 512)] * ((n + 511) // 512)
    assert sum(widths) == n
    bufs = dict({"xr": 2, "xxl": 10, "hr1": 3, "pp": 12, "fp": 3, "x3": 3, "z0": 1}, **(bufs or {}))
    nc = bacc.Bacc("TRN2")
    xin = nc.dram_tensor("xin", [XROWS, n], BF16, kind="ExternalInput")
    xx2 = nc.dram_tensor("xx2", [78, n], BF16, kind="ExternalInput")
    xr3 = nc.dram_tensor("xr3", [117, n], BF16, kind="ExternalInput")
    xpk = nc.dram_tensor("xpk", [117, 8 * FILLW0], BF16, kind="ExternalInput")
    xpk1 = nc.dram_tensor("xpk1", [117, 8 * FILLW1], BF16, kind="ExternalInput")
    xrpk = nc.dram_tensor("xrpk", [128, NBI * FILLW0], BF16, kind="ExternalInput")

    w0 = nc.dram_tensor("w0", [128, NCH0 * 128], BF16, kind="ExternalInput")
    w1 = nc.dram_tensor("w1", [NBI * NBJ1, 128, 128], BF16, kind="ExternalInput")
    wts = nc.dram_tensor("wts", [128, 2], BF16, kind="ExternalInput")
    out = nc.dram_tensor("out", [1, n], F32, kind="ExternalOutput")
    h1scr = nc.dram_tensor("h1scr", [64, n], BF16, kind="Internal")

    def tt_eng(idx):
        return getattr(nc, tt_engines[idx % len(tt_engines)])

    def tt_eng2(layer, c):
        # with a 27-element tuple, free per-chunk assignment: l0 c in 0..6,
        # l1 c in 7..26; otherwise fall back to the baseline 9-cycle
        if len(tt_engines) == 27:
            return getattr(nc, tt_engines[c if layer == 0 else 7 + c])
        return tt_eng(c if layer == 0 else c + 1)

    with TileContext(nc) as tc:
        with (
            tc.tile_pool(name="const", bufs=1) as cpool,
            tc.tile_pool(name="xr", bufs=bufs["xr"]) as xrpool,
            tc.tile_pool(name="xxl", bufs=bufs["xxl"]) as xxlpool,
            tc.tile_pool(name="hr1", bufs=bufs["hr1"]) as hr1pool,
            tc.tile_pool(name="pp", bufs=bufs["pp"]) as ppool,
            tc.tile_pool(name="fp", bufs=bufs["fp"]) as fpool,
            tc.tile_pool(name="zp", bufs=1, space="PSUM") as zpool,
            tc.tile_pool(name="mp", bufs=2, space="PSUM") as mpool,
        ):
            w0sb = cpool.tile([128, NCH0 * 128], BF16, tag="w0sb")
            w1sb = cpool.tile([128, NBI * NBJ1 * 128], BF16, tag="w1sb")
            wtsb = cpool.tile([128, 2], BF16, tag="wtsb")
            wt0sb = wtsb[0:128, 0:1]
            wt1sb = wtsb[0:128, 1:2]

            def load_weights():
                # one DMA per weight array: iterate (k, c, m) so the SBUF dst
                # is partition-major with only within-partition inner dims
                nc.sync.dma_start(w0sb[:], w0[:])
                nc.sync.dma_start(wtsb[:], wts[:])

            def load_w1():
                nch1 = NBI * NBJ1
                src1 = bass.AP(w1[:].tensor, 0, [[128, 128], [16384, nch1], [1, 128]])
                dst1 = bass.AP(w1sb[:].tensor, w1sb[:].offset,
                               [[nch1 * 128, 128], [128, nch1], [1, 128]])
                nc.sync.dma_start(dst1, src1)

            T = len(widths)
            starts = [sum(widths[:i]) for i in range(T)]
            st = {}  # per-tile state carried t -> t+1

            def stage_l0_dma(t):
                nt = widths[t]
                cs = starts[t]
                if t == 0:
                    # tile 0: one contiguous host-packed DMA for all 8
                    # x-operand blocks -- the fill is HWDGE-start-bound
                    assert nt == FILLW0 and cs == 0
                    xp = xxlpool.tile([128, 8 * nt], BF16, tag=f"xpkt{t}",
                                      bufs=1)
                    nc.sync.dma_start(xp[0:117, :], xpk[:, :])
                    xxsl = []
                    for c in range(NCH0):
                        rows = min(3, 20 - 3 * c) * 39
                        xxsl.append(xp[0:rows, (1 + c) * nt:(2 + c) * nt])
                    st[t] = {"xr3t": xp[0:128, 0:nt], "xxsl": xxsl}
                    return
                xr3t = xxlpool.tile([128, nt], BF16, tag="xr3t", bufs=bufs["x3"])
                nc.sync.dma_start(xr3t[0:117, :], xr3[:, cs:cs + nt])
                if t < 0:
                    # fill tiles: batched group loads (3 starts for 7 chunks)
                    # -- the fill is HWDGE-start-bound at ~650ns per start
                    xxsl = []
                    for g, (c0, ncg) in enumerate(((0, 3), (3, 3), (6, 1))):
                        xxl = xxlpool.tile([128, ncg * nt], BF16,
                                           tag=f"xxlg{g}", bufs=1)
                        srcg = bass.AP(xx2[:].tensor, 3 * c0 * n + cs,
                                       [[3 * n, ncg], [n, 3], [n, 39], [1, nt]])
                        dstg = bass.AP(xxl[:].tensor, xxl[:].offset,
                                       [[nt, ncg], [39 * ncg * nt, 3],
                                        [ncg * nt, 39], [1, nt]])
                        nc.sync.dma_start(dstg, srcg)
                        for k in range(ncg):
                            c = c0 + k
                            rows = min(3, 20 - 3 * c) * 39
                            xxsl.append(xxl[0:rows, k * nt:k * nt + nt])
                    st[t] = {"xr3t": xr3t, "xxsl": xxsl}
                    return
                xxsl = []
                for c in range(NCH0):
                    ng = min(3, 20 - 3 * c)
                    rows = ng * 39
                    xxl = xxlpool.tile([128, nt], BF16, tag="xxl")
                    src = bass.AP(xx2[:].tensor, (3 * c) * n + cs,
                                  [[n, ng], [n, 39], [1, nt]])
                    nc.sync.dma_start(
                        bass.AP(xxl[:].tensor, xxl[:].offset, [[nt, rows], [1, nt]]),
                        src)
                    xxsl.append(xxl[0:rows, :])
                st[t] = {"xr3t": xr3t, "xxsl": xxsl}

            def stage_l0_compute(t):
                nt = widths[t]
                npieces = (nt + 511) // 512
                xr3t = st[t]["xr3t"]
                xxls = st[t]["xxsl"]
                z0 = zpool.tile([128, nt], F32, tag="z0", bufs=bufs.get("z0", 1),
                                padded_shape=[128, max(widths)])
                for c in range(NCH0):
                    ng = min(3, 20 - 3 * c)
                    rows = ng * 39
                    p = ppool.tile([128, nt], BF16, tag="p")
                    tt_eng2(0, c).tensor_tensor(p[0:rows, :],
                                            xr3t[0:rows, 0:nt],
                                            xxls[c], AluOpType.mult)
                    for q in range(npieces):
                        pw = min(512, nt - q * 512)
                        nc.tensor.matmul(
                            z0[:, q * 512:q * 512 + pw],
                            w0sb[0:rows, c * 128:(c + 1) * 128],
                            p[0:rows, q * 512:q * 512 + pw],
                            start=(c == 0), stop=(c == NCH0 - 1))
                st[t]["z0"] = z0

            def load_xr(t):
                nt = widths[t]
                cs = starts[t]
                if t == 0:
                    # host-packed replicated x for tile 0, two starts
                    xt = xrpool.tile([128, NBI * nt], BF16, tag="xrpk0",
                                     bufs=1)
                    nc.sync.dma_start(
                        bass.AP(xt[:].tensor, xt[:].offset,
                                [[NBI * nt, 128], [1, 3 * nt]]),
                        bass.AP(xrpk[:].tensor, 0,
                                [[NBI * FILLW0, 128], [1, 3 * nt]]))
                    nc.sync.dma_start(
                        bass.AP(xt[:].tensor, xt[:].offset + 3 * nt,
                                [[NBI * nt, 128], [1, 2 * nt]]),
                        bass.AP(xrpk[:].tensor, 3 * nt,
                                [[NBI * FILLW0, 128], [1, 2 * nt]]))
                    st[t]["xr"] = [xt[0:128, ib * nt:(ib + 1) * nt]
                                   for ib in range(NBI)]
                    return
                xr = []
                for ib in range(NBI):
                    xt = xrpool.tile([128, nt], BF16, tag=f"xr{ib}")
                    src = bass.AP(xin[:].tensor, (A * ib) * n + cs,
                                  [[n, A], [0, G], [1, nt]])
                    nc.sync.dma_start(xt[:], src)
                    xr.append(xt[:])
                st[t]["xr"] = xr

            def stage_h1(t):
                nt = widths[t]
                cs = starts[t]
                load_xr(t)
                z0 = st[t]["z0"]
                f01 = fpool.tile([128, nt], BF16, tag="f01")
                nc.scalar.activation(f01[0:128, :], z0[0:128, :],
                                     mybir.ActivationFunctionType.Relu)
                h1 = f01[0:64, :]
                deng = nc.sync if t == 0 else nc.scalar
                deng.dma_start(h1scr[:, cs:cs + nt], h1)
                hr1 = []
                for jb in range(NBJ1):
                    ht = hr1pool.tile([128, nt], BF16, tag=f"hr1{jb}")
                    src = bass.AP(h1scr[:].tensor, (G * jb) * n + cs,
                                  [[0, A], [n, G], [1, nt]])
                    deng.dma_start(ht[:], src)
                    hr1.append(ht)
                st[t]["f01"] = f01
                st[t]["hr1"] = hr1

            def stage_l1(t):
                nt = widths[t]
                npieces = (nt + 511) // 512
                xr = st[t]["xr"]
                f01 = st[t]["f01"]
                hr1 = st[t]["hr1"]
                z1 = zpool.tile([128, nt], F32, tag="z1")
                nchunk1 = NBI * NBJ1
                for ib in range(NBI):
                    for jb in range(NBJ1):
                        c = ib * NBJ1 + jb
                        p = ppool.tile([128, nt], BF16, tag="p")
                        tt_eng2(1, c).tensor_tensor(p[:], xr[ib], hr1[jb][:], AluOpType.mult)
                        for q in range(npieces):
                            pw = min(512, nt - q * 512)
                            nc.tensor.matmul(
                                z1[:, q * 512:q * 512 + pw],
                                w1sb[:, c * 128:(c + 1) * 128],
                                p[:, q * 512:q * 512 + pw],
                                start=(c == 0), stop=(c == nchunk1 - 1))
                f1 = fpool.tile([128, nt], BF16, tag="f1")
                nc.scalar.activation(f1[:], z1[:], mybir.ActivationFunctionType.Relu)
                # finals: PE matvec per 512-piece, DVE grouped d-reduce into acc
                cs1 = starts[t]
                mvs = fpool.tile([1, nt], F32, tag="mvs", bufs=3)
                for q in range(npieces):
                    pw = min(512, nt - q * 512)
                    mv = mpool.tile([1, pw], F32, tag="mv", padded_shape=[1, 512])
                    nc.tensor.matmul(mv[0:1, :], wt0sb,
                                     f01[:, q * 512:q * 512 + pw], start=True, stop=False)
                    nc.tensor.matmul(mv[0:1, :], wt1sb,
                                     f1[:, q * 512:q * 512 + pw], start=False, stop=True)
                    nc.scalar.activation(mvs[0:1, q * 512:q * 512 + pw], mv[0:1, :],
                                         mybir.ActivationFunctionType.Copy)
                nc.scalar.dma_start(out[0:1, cs1:cs1 + nt], mvs[0:1, :])
                del st[t]

            stage_l0_dma(0)
            load_weights()
            if T > 1:
                stage_l0_dma(1)
            load_w1()
            for t in range(T):
                stage_l0_compute(t)
                if t > 0:
                    stage_l1(t - 1)
                stage_h1(t)
                if t + 2 < T:
                    stage_l0_dma(t + 2)
            stage_l1(T - 1)
    nc.compile()
    return nc


def _prep_weights(f0, f1, dense_w):
    w0r = np.asarray(f0, np.float32).reshape(F0, F0, 128)
    # folded: w0f[o, i] = W0[i, (i+o)%39] + (o>0) * W0[(i+o)%39, i], o in 0..19
    w0f = np.zeros((20, F0, 128), np.float32)
    for o in range(20):
        i = np.arange(F0)
        j = (i + o) % F0
        w0f[o] = w0r[i, j]
        if o > 0:
            w0f[o] += w0r[j, i]
    w0b = np.zeros((NCH0, 128, 128), np.float32)
    for c in range(NCH0):
        ng = min(3, 20 - 3 * c)
        w0b[c, :ng * F0] = w0f[3 * c:3 * c + ng].reshape(ng * F0, 128)
    w1r = np.asarray(f1, np.float32).reshape(F0, FK1, 128)
    w1p = np.zeros((XROWS, FK1, 128), np.float32)
    w1p[:F0] = w1r
    w1b = np.zeros((NBI * NBJ1, 128, 128), np.float32)
    r = np.arange(128)
    for ib in range(NBI):
        for jb in range(NBJ1):
            w1b[ib * NBJ1 + jb] = w1p[A * ib + r // G, G * jb + r % G, :]
    bf = ml_dtypes.bfloat16
    dw = np.asarray(dense_w, np.float32)
    wts = np.concatenate([np.concatenate([np.zeros((64, 1), np.float32),
                                          dw[0:64]]),
                          np.ascontiguousarray(dw[64:192])], axis=1)
    w0flat = np.ascontiguousarray(w0b.transpose(1, 0, 2).reshape(128, NCH0 * 128))
    return {"w0": w0flat.astype(bf), "w1": w1b.astype(bf),
            "wts": wts.astype(bf)}


def _prep_x(xc):
    bc = xc.shape[0]
    xt = np.transpose(np.asarray(xc, np.float32), (1, 0, 2)).reshape(F0, bc * D)
    xp = np.zeros((XROWS, bc * D), np.float32)
    xp[:F0] = xt
    xb = xp.astype(ml_dtypes.bfloat16)
    xx2 = np.concatenate([xb[:F0], xb[:F0]], axis=0)
    xr3 = np.tile(xb[:F0], (3, 1))
    def packx(cs, w):
        blocks = [xr3[:, cs:cs + w]]
        for c in range(NCH0):
            ch = np.concatenate([xx2[o:o + 39, cs:cs + w]
                                 for o in (3 * c, 3 * c + 1, 3 * c + 2)], axis=0)
            blocks.append(ch)
        return np.concatenate(blocks, axis=1)
    xpk = packx(0, FILLW0)
    xpk1 = packx(FILLW0, FILLW1)
    r = np.arange(128)
    xrpk = np.concatenate([xb[A * ib + r // G, 0:FILLW0]
                           for ib in range(NBI)], axis=1)
    r = np.arange(128)
    reps = [xb[A * ib + r // G, 0:FILLW0] for ib in range(NBI)]
    xrpk = np.concatenate(reps, axis=1)
    return xb, xx2, xr3, xpk, xpk1, xrpk


_cache = {}
last_results = None


def _get_nc():
    if "nc" not in _cache:
        _cache["nc"] = _build()
    return _cache["nc"]


def kernel(x, f0, f1, dense_w, dense_b):
    nc = _get_nc()
    common = _prep_weights(f0, f1, dense_w)
    x = np.asarray(x, np.float32)
    in_maps = []
    for c in range(NCORES):
        m = dict(common)
        (m["xin"], m["xx2"], m["xr3"], m["xpk"], m["xpk1"],
         m["xrpk"]) = _prep_x(x[c * BC:(c + 1) * BC])
        in_maps.append(m)
    import os
    trace = bool(os.environ.get("CIN_TRACE"))
    res = run_bass_kernel_spmd(nc, in_maps, core_ids=list(range(NCORES)),
                               trace=trace)
    global last_results
    last_results = res
    out = np.concatenate(
        [r["out"][0].reshape(BC, D).sum(axis=1) for r in res.results])
    return out.astype(np.float32).reshape(B, 1) + np.asarray(dense_b, np.float32)[None, :]


def bench(x, f0, f1, dense_w, dense_b, iters=50):
    """Steady-state per-iteration device time (ns) via repeated execution."""
    import time
    import jax
    import jax.numpy as jnp
    from jax.sharding import Mesh, PartitionSpec
    from jax.experimental.shard_map import shard_map
    import concourse.bass2jax as b2j
    import concourse.mybir as mybir

    nc = _get_nc()
    b2j.install_neuronx_cc_hook()
    common = _prep_weights(f0, f1, dense_w)
    x = np.asarray(x, np.float32)
    in_maps = []
    for c in range(NCORES):
        m = dict(common)
        (m["xin"], m["xx2"], m["xr3"], m["xpk"], m["xpk1"],
         m["xrpk"]) = _prep_x(x[c * BC:(c + 1) * BC])
        in_maps.append(m)

    partition_name = nc.partition_id_tensor.name if nc.partition_id_tensor else None
    in_names, out_names, out_avals, zero_outs = [], [], [], []
    for alloc in nc.m.functions[0].allocations:
        if not isinstance(alloc, mybir.MemoryLocationSet):
            continue
        name = alloc.memorylocations[0].name
        if alloc.kind == "ExternalInput":
            if name != partition_name:
                in_names.append(name)
        elif alloc.kind == "ExternalOutput":
            out_names.append(name)
            shape = tuple(alloc.tensor_shape)
            dtype = mybir.dt.np(alloc.dtype)
            out_avals.append(jax.core.ShapedArray(shape, dtype))
            zero_outs.append(np.zeros(shape, dtype))
    n_params = len(in_names)
    all_names = in_names + out_names
    if partition_name is not None:
        all_names = all_names + [partition_name]

    def _body(*args):
        operands = list(args)
        if partition_name is not None:
            operands.append(b2j.partition_id_tensor())
        outs = b2j._bass_exec_p.bind(
            *operands, out_avals=tuple(out_avals), in_names=tuple(all_names),
            out_names=tuple(out_names), lowering_input_output_aliases=(),
            sim_require_finite=False, sim_require_nnan=False, nc=nc)
        return tuple(outs)

    devices = jax.devices()[:NCORES]
    mesh = Mesh(np.asarray(devices), ("core",))
    nin = n_params + len(out_names)
    f = jax.jit(shard_map(_body, mesh=mesh,
                          in_specs=(PartitionSpec("core"),) * nin,
                          out_specs=(PartitionSpec("core"),) * len(out_names),
                          check_rep=False))
    concat_in = [np.concatenate([np.asarray(in_maps[c][nm]) for c in range(NCORES)], axis=0)
                 for nm in in_names] + \
                [np.concatenate([z for _ in range(NCORES)], axis=0) for z in zero_outs]
    dev_in = [jax.device_put(a) for a in concat_in]
    r = f(*dev_in)
    jax.block_until_ready(r)
    def timed(n):
        t0 = time.perf_counter()
        for _ in range(n):
            rr = f(*dev_in)
        jax.block_until_ready(rr)
        return time.perf_counter() - t0
    timed(20)
    t1 = timed(iters)
    t2 = timed(2 * iters)
    slope = (t2 - t1) / iters
    print(f"bench: avg@{iters}={t1/iters*1e6:.0f}us avg@{2*iters}={t2/(2*iters)*1e6:.0f}us slope={slope*1e6:.0f}us")
    return slope * 1e9

